# revision 6
# baseline (speedup 1.0000x reference)
"""Fused single-launch BPCA pooling v2.

Per core: 4 samples. Per sample:
  - DMA 4 chunks [128, 4096] f32 (16 KiB/partition contiguous lines).
  - Gram in float32r with 256-wide moving operand (1 cycle/row on the PE,
    vs 4 for plain fp32): G256 accumulated in two PSUM tiles [128, 256].
  - Extraction: mask+strided-reduce -> [128,4] halves, PE-fold to S [4,4],
    spread to replicated Sflat [128, 16].
  - Top eigenvector: Gershgorin-shifted power iteration by repeated
    squaring (8 squarings, trace renorms), all replicated-flat DVE ops.
  - Projection: 4 scalar_tensor_tensor passes per chunk (stride-4 plane
    views); 3 chunks on vector, 1 on gpsimd. No mean-centering on device:
    host folds -mu.v and 1/||v|| and the LAPACK sign into one post-scale.

Host supplies per-sample means (aux, for the cov shift) and the fixed
power-iteration seed vector; device returns S and v per sample in stats.
"""

import numpy as np
from contextlib import ExitStack

import concourse.bass as bass
import concourse.tile as tile
from concourse import bacc, mybir
from concourse.bass_utils import run_bass_kernel_spmd

B, H, W, C = 32, 64, 64, 512
N_CORES = 8
BPC = B // N_CORES
SAMPLE = H * W * C
NROWS = SAMPLE // 4
OUT_SAMPLE = SAMPLE // 4
F32 = mybir.dt.float32
F32R = mybir.dt.float32r
I32 = mybir.dt.int32
ALU = mybir.AluOpType
AF = mybir.ActivationFunctionType
AXL = mybir.AxisListType

NSQ = 8                       # squarings; contamination ~ ratio^-256
RENORM_ITS = (1, 3, 5)        # renorm by trace after these squarings
EVEC = [0.9129, -0.6011, 0.3683, 1.0577]   # fixed generic seed vector


def _in_dram_ap(x, b, half, q):
    off = b * SAMPLE + half * 32768 + q * 4096
    return bass.AP(x, off, [[65536, 32], [8192, 4], [1, 4096]])


def _v(ap, axes, extra_off=0):
    """Free-dim view of a [P, F] tile AP with custom free axes."""
    return bass.AP(ap.tensor, ap.offset + extra_off, [list(ap.ap[0])] + axes)


def _build_fused():
    nc = bacc.Bacc("TRN2", target_bir_lowering=False, debug=False)
    x = nc.dram_tensor("x", [BPC * SAMPLE], F32, kind="ExternalInput")
    AUXW = 8 * BPC + 8
    aux = nc.dram_tensor("aux", [128, AUXW], F32, kind="ExternalInput")
    y = nc.dram_tensor("y", [BPC * OUT_SAMPLE], F32, kind="ExternalOutput")
    st = nc.dram_tensor("stats", [BPC, 20], F32, kind="ExternalOutput")

    with tile.TileContext(nc) as tc, ExitStack() as ctx:
        const = ctx.enter_context(tc.tile_pool(name="const", bufs=1))
        chunks = ctx.enter_context(tc.tile_pool(name="chunks", bufs=10))
        psumg = ctx.enter_context(tc.tile_pool(name="psumg", bufs=2, space="PSUM"))
        psums = ctx.enter_context(tc.tile_pool(name="psums", bufs=2, space="PSUM"))
        red = ctx.enter_context(tc.tile_pool(name="red", bufs=2))
        eig = ctx.enter_context(tc.tile_pool(name="eig", bufs=2))
        ycp = ctx.enter_context(tc.tile_pool(name="ycp", bufs=5))

        state = {}

        def build_consts():
            auxp = const.tile([128, AUXW], F32)
            nc.sync.dma_start(auxp[:], bass.AP(aux, 0, [[AUXW, 128], [1, AUXW]]))
            pidx_i = const.tile([128, 1], I32)
            nc.gpsimd.iota(pidx_i[:], [[0, 1]], base=0, channel_multiplier=1)
            pmod_i = const.tile([128, 1], I32)
            nc.vector.tensor_scalar(pmod_i[:], pidx_i[:], 3, None, ALU.bitwise_and)
            pgrp_i = const.tile([128, 1], I32)
            nc.vector.tensor_scalar(pgrp_i[:], pidx_i[:], -4, None, ALU.bitwise_and)
            E4_i = const.tile([128, 4], I32)
            for k in range(4):
                nc.vector.tensor_scalar(E4_i[:, k:k + 1], pmod_i[:], k, None,
                                        ALU.is_equal)
            E4 = const.tile([128, 4], F32)
            nc.vector.tensor_copy(E4[:], E4_i[:])
            # masks [128, 256] for G256 halves:
            #   maskA[p, n] = ((n>>2) == (p>>2)); maskB: (n>>2) == 32 + (p>>2)
            cidx_i = const.tile([128, 256], I32)
            nc.gpsimd.iota(cidx_i[:], [[1, 256]], base=0, channel_multiplier=0)
            cgrp_i = const.tile([128, 256], I32)
            nc.vector.tensor_scalar(cgrp_i[:], cidx_i[:], -4, None, ALU.bitwise_and)
            cgrp = const.tile([128, 256], F32)
            nc.vector.tensor_copy(cgrp[:], cgrp_i[:])
            pgrp = const.tile([128, 1], F32)
            nc.vector.tensor_copy(pgrp[:], pgrp_i[:])
            pgrpB = const.tile([128, 1], F32)
            nc.vector.tensor_scalar(pgrpB[:], pgrp[:], 128.0, None, ALU.add)
            maskA = const.tile([128, 256], F32)
            nc.vector.tensor_scalar(maskA[:], cgrp[:], pgrp[:], 0.0,
                                    ALU.subtract, ALU.is_equal)
            maskB = const.tile([128, 256], F32)
            nc.vector.tensor_scalar(maskB[:], cgrp[:], pgrpB[:], 0.0,
                                    ALU.subtract, ALU.is_equal)
            # FM[p, u] = (u//4 == p), u in [0,16)  (only partitions 0..3 used)
            uidx_i = const.tile([128, 16], I32)
            nc.gpsimd.iota(uidx_i[:], [[1, 16]], base=0, channel_multiplier=0)
            ugrp_i = const.tile([128, 16], I32)
            nc.vector.tensor_scalar(ugrp_i[:], uidx_i[:], -4, None, ALU.bitwise_and)
            ugrp = const.tile([128, 16], F32)
            nc.vector.tensor_copy(ugrp[:], ugrp_i[:])
            pidx4 = const.tile([128, 1], F32)
            nc.vector.tensor_copy(pidx4[:], pidx_i[:])
            nc.vector.tensor_scalar(pidx4[:], pidx4[:], 4.0, None, ALU.mult)
            FM = const.tile([128, 16], F32)
            nc.vector.tensor_scalar(FM[:], ugrp[:], pidx4[:], 0.0,
                                    ALU.subtract, ALU.is_equal)
            # dm16[p, u] = (u//4 == u%4): flat 4x4 identity
            umod_i = const.tile([128, 16], I32)
            nc.vector.tensor_scalar(umod_i[:], uidx_i[:], 3, None, ALU.bitwise_and)
            ud_i = const.tile([128, 16], I32)
            nc.vector.tensor_scalar(ud_i[:], ugrp_i[:], 2, None,
                                    ALU.arith_shift_right)
            umod = const.tile([128, 16], F32)
            nc.vector.tensor_copy(umod[:], umod_i[:])
            ud = const.tile([128, 16], F32)
            nc.vector.tensor_copy(ud[:], ud_i[:])
            dm16 = const.tile([128, 16], F32)
            nc.vector.tensor_tensor(dm16[:], ud[:], umod[:], ALU.is_equal)
            ones4x128 = const.tile([4, 128], F32)
            nc.vector.memset(ones4x128[:], 1.0)
            state.update(auxp=auxp, E4=E4, maskA=maskA, maskB=maskB, FM=FM,
                         dm16=dm16, ones4x128=ones4x128)

        def emit_gram(b, ctiles):
            psA = psumg.tile([128, 256], F32, tag="psA")
            psB = psumg.tile([128, 256], F32, tag="psB")
            nmm = 0
            for ci, t in enumerate(ctiles):
                for sb in range(16):
                    rhs = t[:, sb * 256:(sb + 1) * 256]
                    lhsA = t[:, sb * 256:sb * 256 + 128]
                    lhsB = t[:, sb * 256 + 128:(sb + 1) * 256]
                    first = nmm == 0
                    last = nmm == 63
                    nc.tensor.matmul(psA[:], lhsA, rhs, start=first, stop=last)
                    nc.tensor.matmul(psB[:], lhsB, rhs, start=first, stop=last)
                    nmm += 1
            return psA, psB

        def emit_extract_eigen(b, psA, psB):
            auxp = state["auxp"]
            dm16 = state["dm16"]
            # ---- extraction: S44 = sum_g G256[4g+k, 4g+l] ----
            mA = red.tile([128, 256], F32, tag="mA")
            nc.vector.tensor_mul(mA[:], psA[:], state["maskA"][:])
            mB = red.tile([128, 256], F32, tag="mB")
            nc.vector.tensor_mul(mB[:], psB[:], state["maskB"][:])
            m4A = red.tile([128, 4], F32, tag="m4A")
            nc.vector.tensor_reduce(m4A[:], _v(mA[:], [[1, 4], [4, 64]]),
                                    AXL.X, ALU.add)
            m4B = red.tile([128, 4], F32, tag="m4B")
            nc.vector.tensor_reduce(m4B[:], _v(mB[:], [[1, 4], [4, 64]]),
                                    AXL.X, ALU.add)
            psE = psums.tile([4, 4], F32, tag="psE")
            nc.tensor.matmul(psE[:], state["E4"][:], m4A[:], start=True, stop=False)
            nc.tensor.matmul(psE[:], state["E4"][:], m4B[:], start=False, stop=True)
            # spread S [4,4] -> [4,16] rows, replicate to [128,16]
            Fm16 = red.tile([4, 16], F32, tag="Fm16")
            s_b = _v(psE[:], [[0, 4], [1, 4]])
            nc.vector.tensor_tensor(Fm16[:].rearrange("p (j l) -> p j l", j=4),
                                    s_b,
                                    state["FM"][0:4, :].rearrange(
                                        "p (j l) -> p j l", j=4),
                                    ALU.mult)
            psS = psums.tile([128, 16], F32, tag="psS")
            nc.tensor.matmul(psS[:], state["ones4x128"][:], Fm16[:],
                             start=True, stop=True)

            # ---- eigen: replicated-flat on [128, 16] ----
            murow = auxp[:, 8 * b:8 * b + 4]
            evec = auxp[:, 8 * BPC:8 * BPC + 4]
            mmf = eig.tile([128, 16], F32, tag="mmf")
            mu_i = _v(murow, [[1, 4], [0, 4]])
            mu_j = _v(murow, [[0, 4], [1, 4]])
            nc.vector.tensor_tensor(mmf[:].rearrange("p (k l) -> p k l", k=4),
                                    mu_i, mu_j, ALU.mult)
            covf = eig.tile([128, 16], F32, tag="covf")
            nc.vector.scalar_tensor_tensor(covf[:], psS[:], 1.0 / NROWS, mmf[:],
                                           ALU.mult, ALU.subtract)
            # -tr/4
            trqn = eig.tile([128, 1], F32, tag="trqn")
            nc.vector.tensor_reduce(trqn[:], _v(covf[:], [[5, 4]]), AXL.X,
                                    ALU.add, negate=True)
            nc.vector.tensor_scalar(trqn[:], trqn[:], 0.25, None, ALU.mult)
            # B0 = covf - (tr/4) I
            B0 = eig.tile([128, 16], F32, tag="B0")
            nc.vector.scalar_tensor_tensor(B0[:], dm16[:], trqn[:], covf[:],
                                           ALU.mult, ALU.add)
            # Gershgorin shift: r = max_i sum_j |B0_ij|
            absr = eig.tile([128, 4], F32, tag="absr")
            nc.vector.tensor_reduce(absr[:].rearrange("p (i u) -> p i u", i=4),
                                    B0[:].rearrange("p (i j) -> p i j", i=4),
                                    AXL.X, ALU.add, apply_absolute_value=True)
            rsh = eig.tile([128, 1], F32, tag="rsh")
            nc.vector.tensor_reduce(rsh[:], absr[:], AXL.X, ALU.max)
            Bc = eig.tile([128, 16], F32, tag="Bc")
            nc.vector.scalar_tensor_tensor(Bc[:], dm16[:], rsh[:], B0[:],
                                           ALU.mult, ALU.add)
            # squarings with ping-pong buffers
            Cc = eig.tile([128, 16], F32, tag="Cc")
            prod = eig.tile([128, 64], F32, tag="prod")
            cur, nxt = Bc, Cc
            for it in range(NSQ):
                nc.vector.tensor_tensor(
                    prod[:].rearrange("p (i j k) -> p i j k", i=4, j=4),
                    _v(cur[:], [[4, 4], [0, 4], [1, 4]]),
                    _v(cur[:], [[0, 4], [1, 4], [4, 4]]),
                    ALU.mult)
                nc.vector.tensor_reduce(
                    nxt[:].rearrange("p (i j) -> p i j", i=4),
                    prod[:].rearrange("p (i j k) -> p i j k", i=4, j=4),
                    AXL.X, ALU.add)
                cur, nxt = nxt, cur
                if it in RENORM_ITS:
                    trc = eig.tile([128, 1], F32, tag="trc")
                    nc.vector.tensor_reduce(trc[:], _v(cur[:], [[5, 4]]),
                                            AXL.X, ALU.add)
                    nc.vector.reciprocal(trc[:], trc[:])
                    nc.vector.tensor_scalar(cur[:], cur[:], trc[:], None,
                                            ALU.mult)
            # v = B @ e  (replicated)
            vprod = eig.tile([128, 16], F32, tag="vprod")
            nc.vector.tensor_tensor(
                vprod[:].rearrange("p (i j) -> p i j", i=4),
                _v(cur[:], [[4, 4], [1, 4]]), _v(evec, [[0, 4], [1, 4]]),
                ALU.mult)
            v_rep = eig.tile([128, 4], F32, tag="v_rep")
            nc.vector.tensor_reduce(
                v_rep[:].rearrange("p (i u) -> p i u", i=4),
                vprod[:].rearrange("p (i j) -> p i j", i=4), AXL.X, ALU.add)
            # stats out: [1, 20] = Sflat | v_dev
            stt = eig.tile([1, 20], F32, tag="stt")
            nc.vector.tensor_copy(stt[:, 0:16], psS[0:1, :])
            nc.vector.tensor_copy(stt[:, 16:20], v_rep[0:1, :])
            nc.sync.dma_start(bass.AP(st, b * 20, [[20, 1], [1, 20]]), stt[:])
            return v_rep

        def emit_proj(pb, pctiles, pv_rep):
            for ci, t in enumerate(pctiles):
                half, q = divmod(ci, 2)
                eng = nc.vector
                yc = ycp.tile([128, 1024], F32, tag="yc", name=f"yc_{pb}_{ci}")
                planes = [_v(t[:].bitcast(F32), [[512, 8], [4, 128]], extra_off=k)
                          for k in range(4)]
                eng.tensor_scalar(yc[:], planes[0], pv_rep[:, 0:1], None,
                                  ALU.mult)
                for k in (1, 2, 3):
                    eng.scalar_tensor_tensor(yc[:], planes[k],
                                             pv_rep[:, k:k + 1], yc[:],
                                             ALU.mult, ALU.add)
                nc.sync.dma_start(
                    bass.AP(y, pb * OUT_SAMPLE + q * 2048 + half * 256,
                            [[16384, 32], [4096, 4], [512, 4], [1, 256]]),
                    yc[:])

        prev = None
        for b in range(BPC):
            ctiles = []
            for ci in range(4):
                half, q = divmod(ci, 2)
                t = chunks.tile([128, 4096], F32R, tag="chunk",
                                name=f"t_{b}_{ci}")
                ctiles.append(t)
                nc.sync.dma_start(t[:], _in_dram_ap(x, b, half, q).bitcast(F32R))
            psA, psB = emit_gram(b, ctiles)
            if b == 0:
                build_consts()
            if prev is not None:
                emit_proj(*prev)
            v_rep = emit_extract_eigen(b, psA, psB)
            prev = (b, ctiles, v_rep)
        emit_proj(*prev)
    nc.compile()
    return nc


_CACHE = {}


def _get(name, builder):
    if name not in _CACHE:
        _CACHE[name] = builder()
    return _CACHE[name]


def make_aux(mean):
    """mean: [BPC, 4] float -> aux array [128, 8*BPC+8]."""
    auxv = np.zeros((128, 8 * BPC + 8), np.float32)
    for b in range(BPC):
        auxv[:, 8 * b:8 * b + 4] = mean[b].astype(np.float32)
    auxv[:, 8 * BPC:8 * BPC + 4] = np.asarray(EVEC, np.float32)
    return auxv


def kernel(inputs: np.ndarray) -> np.ndarray:
    xx = np.ascontiguousarray(np.asarray(inputs, dtype=np.float32))
    assert xx.shape == (B, H, W, C), xx.shape
    xf = xx.reshape(N_CORES, BPC * SAMPLE)
    cores = list(range(N_CORES))
    mean = xx.reshape(B, NROWS, 4).mean(axis=1, dtype=np.float64)  # [B, 4]

    nc = _get("fused", _build_fused)
    in_maps = [
        {"x": xf[c], "aux": make_aux(mean[c * BPC:(c + 1) * BPC])} for c in cores
    ]
    r = run_bass_kernel_spmd(nc, in_maps, cores)
    stats = np.stack([r.results[c]["stats"] for c in cores]).reshape(B, 20)
    yv = np.stack([r.results[c]["y"] for c in cores]).reshape(B, OUT_SAMPLE)

    S = stats[:, 0:16].reshape(B, 4, 4).astype(np.float64)
    v_dev = stats[:, 16:20].astype(np.float64)
    cov = (S / NROWS - np.einsum("bi,bj->bij", mean, mean)).astype(np.float32)

    import jax
    import jax.numpy as jnp
    with jax.default_device(jax.devices("cpu")[0]):
        _, vecs = jnp.linalg.eigh(jnp.asarray(cov))
    v_ref = np.asarray(vecs)[:, :, -1].astype(np.float64)

    dot = (v_ref * v_dev).sum(1)
    scale = np.sign(dot) / np.linalg.norm(v_dev, axis=1)
    offs = -(mean * v_dev).sum(1) * scale          # fold -mu.v into host
    yv = (yv * scale[:, None] + offs[:, None]).astype(np.float32)
    return yv.reshape(B, H // 2, W // 2, C)


# revision 11
# speedup vs baseline: 1.1293x; 1.1293x over previous
"""Fused single-launch BPCA pooling v3.

Per core: 4 samples. Per sample:
  - DMA 8 half-chunks [128, 2048] f32 (8 KiB/partition lines).
  - Gram in float32r with 256-wide moving operand (1 cycle/row on the PE):
    G256 accumulated into two PSUM tiles [128, 256].
  - Extraction: mask-mult + strided reduce -> [128,4] halves, PE-fold to
    S [4,4], spread to replicated Sflat [128, 16] (masks/folders from aux).
  - Top eigenvector: Gershgorin-normalized power iteration; the 7
    squarings run as [4,4] PE matmuls with scalar-engine PSUM->SBUF
    copies (eigenvalues normalized into [~0.5, 1] so no renorms needed).
  - Projection y = x . v: per chunk, 3 tensor_scalar products + 1
    scalar-engine activation product, folded by vector + gpsimd adds.
    Mean-centering, 1/||v|| and the LAPACK sign fold into one host-side
    post-scale using the returned stats.

All constant tables (masks, identities, fold matrices, mu mu^T) are
precomputed on the host and shipped in the aux input.
"""

import numpy as np
from contextlib import ExitStack

import concourse.bass as bass
import concourse.tile as tile
from concourse import bacc, mybir
from concourse.bass_utils import run_bass_kernel_spmd

B, H, W, C = 32, 64, 64, 512
N_CORES = 8
BPC = B // N_CORES
SAMPLE = H * W * C
NROWS = SAMPLE // 4
OUT_SAMPLE = SAMPLE // 4
F32 = mybir.dt.float32
F32R = mybir.dt.float32r
ALU = mybir.AluOpType
AF = mybir.ActivationFunctionType
AXL = mybir.AxisListType

NSQ = 7                       # squarings; worst contamination ~3e-5
EVEC = [0.9129, -0.6011, 0.3683, 1.0577]   # fixed generic seed vector

# aux column layout
MMF_OFF = 0                   # 16 per sample: flat mu mu^T
C_E = 16 * BPC                # 1 col: eigen seed, rows 0..3
C_I4 = C_E + 1                # 4 cols: I4, rows 0..3
C_FM = C_I4 + 4               # 16 cols: FM[p,u] = (u//4 == p)
C_DM = C_FM + 16              # 16 cols: dm16 flat identity
C_DMQ = C_DM + 16             # 16 cols: 0.25*dm16
C_E4 = C_DMQ + 16             # 4 cols: E4[p,k] = (p%4 == k)
C_ONES = C_E4 + 4             # 128 cols: ones
C_MA = C_ONES + 128           # 256 cols: maskA
C_MB = C_MA + 256             # 256 cols: maskB
AUXW = C_MB + 256


def _in_dram_ap_half(x, b, half, q, h2):
    off = b * SAMPLE + half * 32768 + q * 4096 + h2 * 2048
    return bass.AP(x, off, [[65536, 32], [8192, 4], [1, 2048]])


def _v(ap, axes, extra_off=0):
    """Free-dim view of a [P, F] tile AP with custom free axes."""
    return bass.AP(ap.tensor, ap.offset + extra_off, [list(ap.ap[0])] + axes)


def _build_fused():
    nc = bacc.Bacc("TRN2", target_bir_lowering=False, debug=False)
    x = nc.dram_tensor("x", [BPC * SAMPLE], F32, kind="ExternalInput")
    aux = nc.dram_tensor("aux", [128, AUXW], F32, kind="ExternalInput")
    y = nc.dram_tensor("y", [BPC * OUT_SAMPLE], F32, kind="ExternalOutput")
    st = nc.dram_tensor("stats", [1, BPC * 20], F32, kind="ExternalOutput")

    with tile.TileContext(nc) as tc, ExitStack() as ctx:
        const = ctx.enter_context(tc.tile_pool(name="const", bufs=1))
        chunks = ctx.enter_context(tc.tile_pool(name="chunks", bufs=10))
        psumg = ctx.enter_context(tc.tile_pool(name="psumg", bufs=2, space="PSUM"))
        psums = ctx.enter_context(tc.tile_pool(name="psums", bufs=1, space="PSUM"))
        red = ctx.enter_context(tc.tile_pool(name="red", bufs=2))
        eig = ctx.enter_context(tc.tile_pool(name="eig", bufs=2))
        plp = ctx.enter_context(tc.tile_pool(name="plp", bufs=2))

        auxp = const.tile([128, AUXW], F32)
        nc.sync.dma_start(auxp[:], bass.AP(aux, 0, [[AUXW, 128], [1, AUXW]]))
        sttile = const.tile([1, BPC * 20], F32)

        def emit_gram_half(t, h2, first, last):
            psA = state["psA"]
            psB = state["psB"]
            for i in range(8):
                sb = h2 * 8 + i
                rhs = t[:, sb * 256:(sb + 1) * 256]
                lhsA = t[:, sb * 256:sb * 256 + 128]
                lhsB = t[:, sb * 256 + 128:(sb + 1) * 256]
                st0 = first and i == 0
                sp = last and i == 7
                nc.tensor.matmul(psA[:], lhsA, rhs, start=st0, stop=sp)
                nc.tensor.matmul(psB[:], lhsB, rhs, start=st0, stop=sp)

        def emit_extract_eigen(b, psA, psB):
            maskA = auxp[:, C_MA:C_MA + 256]
            maskB = auxp[:, C_MB:C_MB + 256]
            dm16 = auxp[:, C_DM:C_DM + 16]
            dm16q = auxp[:, C_DMQ:C_DMQ + 16]
            # ---- extraction: S = sum_g G256[4g+k, 4g+l] ----
            mA = red.tile([128, 256], F32, tag="mA")
            nc.vector.tensor_mul(mA[:], psA[:], maskA)
            mB = red.tile([128, 256], F32, tag="mB")
            nc.vector.tensor_mul(mB[:], psB[:], maskB)
            m4A = red.tile([128, 4], F32, tag="m4A")
            nc.vector.tensor_reduce(m4A[:], _v(mA[:], [[1, 4], [4, 64]]),
                                    AXL.X, ALU.add)
            m4B = red.tile([128, 4], F32, tag="m4B")
            nc.vector.tensor_reduce(m4B[:], _v(mB[:], [[1, 4], [4, 64]]),
                                    AXL.X, ALU.add)
            psE = psums.tile([4, 4], F32, tag="scr", name=f"psE_{b}")
            E4 = auxp[:, C_E4:C_E4 + 4]
            nc.tensor.matmul(psE[:], E4, m4A[:], start=True, stop=False)
            nc.tensor.matmul(psE[:], E4, m4B[:], start=False, stop=True)
            Fm16 = red.tile([4, 16], F32, tag="Fm16")
            s_b = _v(psE[:], [[0, 4], [1, 4]])
            nc.vector.tensor_tensor(Fm16[:].rearrange("p (j l) -> p j l", j=4),
                                    s_b,
                                    _v(auxp[0:4, :], [[4, 4], [1, 4]], C_FM),
                                    ALU.mult)
            psS = psums.tile([128, 16], F32, tag="psS", name=f"psS_{b}")
            nc.tensor.matmul(psS[:], auxp[0:4, C_ONES:C_ONES + 128], Fm16[:],
                             start=True, stop=True)

            # ---- eigen setup (replicated flat [128, 16]) ----
            covf = eig.tile([128, 16], F32, tag="covf")
            nc.vector.scalar_tensor_tensor(
                covf[:], psS[:], 1.0 / NROWS,
                auxp[:, MMF_OFF + 16 * b:MMF_OFF + 16 * b + 16],
                ALU.mult, ALU.subtract)
            trqn = eig.tile([128, 1], F32, tag="trqn")
            nc.vector.tensor_reduce(trqn[:], _v(covf[:], [[5, 4]]), AXL.X,
                                    ALU.add, negate=True)
            B0 = eig.tile([128, 16], F32, tag="B0")
            nc.vector.scalar_tensor_tensor(B0[:], dm16q, trqn[:], covf[:],
                                           ALU.mult, ALU.add)
            absr = eig.tile([128, 4], F32, tag="absr")
            nc.vector.tensor_reduce(absr[:].rearrange("p (i u) -> p i u", i=4),
                                    B0[:].rearrange("p (i j) -> p i j", i=4),
                                    AXL.X, ALU.add, apply_absolute_value=True)
            rsh = eig.tile([128, 1], F32, tag="rsh")
            nc.vector.tensor_reduce(rsh[:], absr[:], AXL.X, ALU.max)
            rrec = eig.tile([128, 1], F32, tag="rrec")
            nc.vector.reciprocal(rrec[:], rsh[:])
            Bc = eig.tile([128, 16], F32, tag="Bc")
            nc.vector.scalar_tensor_tensor(Bc[:], dm16, rsh[:], B0[:],
                                           ALU.mult, ALU.add)
            nc.vector.tensor_scalar(Bc[:], Bc[:], rrec[:], 0.5, ALU.mult,
                                    ALU.mult)
            # ---- squarings as [4,4] PE matmuls ----
            Bpe = eig.tile([4, 4], F32, tag="Bpe")
            nc.sync.dma_start(Bpe[:], _v(Bc[0:1, :], [[1, 16]]))
            cur = Bpe
            for k in range(NSQ):
                psq = psums.tile([4, 4], F32, tag="scr", name=f"psq_{b}_{k}")
                nc.tensor.matmul(psq[:], cur[:], cur[:], start=True, stop=True)
                nxt = eig.tile([4, 4], F32, tag=f"sq{k % 2}")
                nc.scalar.copy(nxt[:], psq[:])
                cur = nxt
            # ---- v = C @ e, replicate across partitions via PE ----
            vps = psums.tile([4, 1], F32, tag="scr", name=f"vps_{b}")
            nc.tensor.matmul(vps[:], cur[:], auxp[0:4, C_E:C_E + 1],
                             start=True, stop=True)
            vdiag = eig.tile([4, 4], F32, tag="vdiag")
            nc.vector.tensor_tensor(vdiag[:], _v(vps[:], [[0, 4]]),
                                    auxp[0:4, C_I4:C_I4 + 4], ALU.mult)
            vrps = psums.tile([128, 4], F32, tag="vrps", name=f"vrps_{b}")
            nc.tensor.matmul(vrps[:], auxp[0:4, C_ONES:C_ONES + 128],
                             vdiag[:], start=True, stop=True)
            v_rep = eig.tile([128, 4], F32, tag="v_rep")
            nc.scalar.copy(v_rep[:], vrps[:])
            # stats slices (batched DMA after last sample)
            nc.scalar.copy(sttile[:, 20 * b:20 * b + 16], psS[0:1, :])
            nc.scalar.copy(sttile[:, 20 * b + 16:20 * b + 20], v_rep[0:1, :])
            return v_rep

        def emit_proj(pb, pctiles, pv_rep):
            for ci, t in enumerate(pctiles):
                half, q = divmod(ci, 2)
                planes = [_v(t[:].bitcast(F32), [[512, 8], [4, 128]],
                             extra_off=k) for k in range(4)]
                pa = plp.tile([128, 1024], F32, tag="pa", name=f"pa{pb}_{ci}")
                pb_ = plp.tile([128, 1024], F32, tag="pb", name=f"pb{pb}_{ci}")
                pc = plp.tile([128, 1024], F32, tag="pc", name=f"pc{pb}_{ci}")
                pd = plp.tile([128, 1024], F32, tag="pd", name=f"pd{pb}_{ci}")
                nc.vector.tensor_scalar(pa[:], planes[0], pv_rep[:, 0:1],
                                        None, ALU.mult)
                nc.vector.tensor_scalar(pb_[:], planes[1], pv_rep[:, 1:2],
                                        None, ALU.mult)
                nc.vector.tensor_scalar(pc[:], planes[2], pv_rep[:, 2:3],
                                        None, ALU.mult)
                nc.scalar.activation(pd[:], planes[3], AF.Identity,
                                     bias=0.0, scale=pv_rep[:, 3:4])
                nc.vector.tensor_tensor(pa[:], pa[:], pb_[:], ALU.add)
                nc.gpsimd.tensor_tensor(pc[:], pc[:], pd[:], ALU.add)
                nc.vector.tensor_tensor(pa[:], pa[:], pc[:], ALU.add)
                nc.sync.dma_start(
                    bass.AP(y, pb * OUT_SAMPLE + q * 2048 + half * 256,
                            [[16384, 32], [4096, 4], [512, 4], [1, 256]]),
                    pa[:])

        state = {}
        prev = None
        for b in range(BPC):
            ctiles = []
            state["psA"] = psumg.tile([128, 256], F32, tag="psA",
                                      name=f"psA_{b}")
            state["psB"] = psumg.tile([128, 256], F32, tag="psB",
                                      name=f"psB_{b}")
            for ci in range(4):
                half, q = divmod(ci, 2)
                t = chunks.tile([128, 4096], F32R, tag="chunk",
                                name=f"t_{b}_{ci}")
                ctiles.append(t)
                for h2 in range(2):
                    nc.sync.dma_start(
                        t[:, h2 * 2048:(h2 + 1) * 2048],
                        _in_dram_ap_half(x, b, half, q, h2).bitcast(F32R))
                    emit_gram_half(t, h2, first=(ci == 0 and h2 == 0),
                                   last=(ci == 3 and h2 == 1))
            if prev is not None:
                emit_proj(*prev)
            v_rep = emit_extract_eigen(b, state["psA"], state["psB"])
            prev = (b, ctiles, v_rep)
        emit_proj(*prev)
        nc.sync.dma_start(bass.AP(st, 0, [[BPC * 20, 1], [1, BPC * 20]]),
                          sttile[:])
    nc.compile()
    return nc


_CACHE = {}


def _get(name, builder):
    if name not in _CACHE:
        _CACHE[name] = builder()
    return _CACHE[name]


def make_aux(mean):
    """mean: [BPC, 4] float -> aux array [128, AUXW]."""
    a = np.zeros((128, AUXW), np.float32)
    p = np.arange(128)
    for b in range(BPC):
        mm = np.outer(mean[b], mean[b]).astype(np.float32).reshape(16)
        a[:, MMF_OFF + 16 * b:MMF_OFF + 16 * b + 16] = mm
    a[0:4, C_E] = np.asarray(EVEC, np.float32)
    a[0:4, C_I4:C_I4 + 4] = np.eye(4, dtype=np.float32)
    u = np.arange(16)
    a[0:4, C_FM:C_FM + 16] = (u[None, :] // 4 == np.arange(4)[:, None])
    a[:, C_DM:C_DM + 16] = ((u // 4) == (u % 4)).astype(np.float32)[None, :]
    a[:, C_DMQ:C_DMQ + 16] = 0.25 * a[:, C_DM:C_DM + 16]
    a[:, C_E4:C_E4 + 4] = (np.arange(4)[None, :] == (p % 4)[:, None])
    a[:, C_ONES:C_ONES + 128] = 1.0
    n = np.arange(256)
    a[:, C_MA:C_MA + 256] = ((n[None, :] >> 2) == (p >> 2)[:, None])
    a[:, C_MB:C_MB + 256] = ((n[None, :] >> 2) == 32 + (p >> 2)[:, None])
    return a


def kernel(inputs: np.ndarray) -> np.ndarray:
    xx = np.ascontiguousarray(np.asarray(inputs, dtype=np.float32))
    assert xx.shape == (B, H, W, C), xx.shape
    xf = xx.reshape(N_CORES, BPC * SAMPLE)
    cores = list(range(N_CORES))
    mean = xx.reshape(B, NROWS, 4).mean(axis=1, dtype=np.float64)  # [B, 4]

    nc = _get("fused", _build_fused)
    in_maps = [
        {"x": xf[c], "aux": make_aux(mean[c * BPC:(c + 1) * BPC])} for c in cores
    ]
    r = run_bass_kernel_spmd(nc, in_maps, cores)
    stats = np.stack([r.results[c]["stats"] for c in cores]).reshape(B, 20)
    yv = np.stack([r.results[c]["y"] for c in cores]).reshape(B, OUT_SAMPLE)

    S = stats[:, 0:16].reshape(B, 4, 4).astype(np.float64)
    v_dev = stats[:, 16:20].astype(np.float64)
    cov = (S / NROWS - np.einsum("bi,bj->bij", mean, mean)).astype(np.float32)

    import jax
    import jax.numpy as jnp
    with jax.default_device(jax.devices("cpu")[0]):
        _, vecs = jnp.linalg.eigh(jnp.asarray(cov))
    v_ref = np.asarray(vecs)[:, :, -1].astype(np.float64)

    dot = (v_ref * v_dev).sum(1)
    scale = np.sign(dot) / np.linalg.norm(v_dev, axis=1)
    offs = -(mean * v_dev).sum(1) * scale          # fold -mu.v into host
    yv = (yv * scale[:, None] + offs[:, None]).astype(np.float32)
    return yv.reshape(B, H // 2, W // 2, C)


# revision 14
# speedup vs baseline: 1.1582x; 1.0255x over previous
"""Fused single-launch BPCA pooling v3.

Per core: 4 samples. Per sample:
  - DMA 8 half-chunks [128, 2048] f32 (8 KiB/partition lines).
  - Gram in float32r with 256-wide moving operand (1 cycle/row on the PE):
    G256 accumulated into two PSUM tiles [128, 256].
  - Extraction: mask-mult + strided reduce -> [128,4] halves, PE-fold to
    S [4,4], spread to replicated Sflat [128, 16] (masks/folders from aux).
  - Top eigenvector: Gershgorin-normalized power iteration; the 7
    squarings run as [4,4] PE matmuls with scalar-engine PSUM->SBUF
    copies (eigenvalues normalized into [~0.5, 1] so no renorms needed).
  - Projection y = x . v: per chunk, 3 tensor_scalar products + 1
    scalar-engine activation product, folded by vector + gpsimd adds.
    Mean-centering, 1/||v|| and the LAPACK sign fold into one host-side
    post-scale using the returned stats.

All constant tables (masks, identities, fold matrices, mu mu^T) are
precomputed on the host and shipped in the aux input.
"""

import numpy as np
from contextlib import ExitStack

import concourse.bass as bass
import concourse.tile as tile
from concourse import bacc, mybir
from concourse.bass_utils import run_bass_kernel_spmd

B, H, W, C = 32, 64, 64, 512
N_CORES = 8
BPC = B // N_CORES
SAMPLE = H * W * C
NROWS = SAMPLE // 4
OUT_SAMPLE = SAMPLE // 4
F32 = mybir.dt.float32
F32R = mybir.dt.float32r
ALU = mybir.AluOpType
AF = mybir.ActivationFunctionType
AXL = mybir.AxisListType

NSQ = 7                       # squarings; worst contamination ~3e-5
EVEC = [0.9129, -0.6011, 0.3683, 1.0577]   # fixed generic seed vector

# aux column layout
MMF_OFF = 0                   # 16 per sample: flat mu mu^T
C_E = 16 * BPC                # 1 col: eigen seed, rows 0..3
C_I4 = C_E + 1                # 4 cols: I4, rows 0..3
C_FM = C_I4 + 4               # 16 cols: FM[p,u] = (u//4 == p)
C_DM = C_FM + 16              # 16 cols: dm16 flat identity
C_DMQ = C_DM + 16             # 16 cols: 0.25*dm16
C_E4 = C_DMQ + 16             # 4 cols: E4[p,k] = (p%4 == k)
C_ONES = C_E4 + 4             # 128 cols: ones
C_MA = C_ONES + 128           # 256 cols: maskA
C_MB = C_MA + 256             # 256 cols: maskB
C_I128 = C_MB + 256           # 128 cols: I128
AUXW = C_I128 + 128


def _in_dram_ap_half(x, b, half, q, h2):
    off = b * SAMPLE + half * 32768 + q * 4096 + h2 * 2048
    return bass.AP(x, off, [[65536, 32], [8192, 4], [1, 2048]])


def _v(ap, axes, extra_off=0):
    """Free-dim view of a [P, F] tile AP with custom free axes."""
    return bass.AP(ap.tensor, ap.offset + extra_off, [list(ap.ap[0])] + axes)


def _build_fused():
    nc = bacc.Bacc("TRN2", target_bir_lowering=False, debug=False)
    x = nc.dram_tensor("x", [BPC * SAMPLE], F32, kind="ExternalInput")
    aux = nc.dram_tensor("aux", [128, AUXW], F32, kind="ExternalInput")
    y = nc.dram_tensor("y", [BPC * OUT_SAMPLE], F32, kind="ExternalOutput")
    st = nc.dram_tensor("stats", [1, BPC * 20], F32, kind="ExternalOutput")

    with tile.TileContext(nc) as tc, ExitStack() as ctx:
        const = ctx.enter_context(tc.tile_pool(name="const", bufs=1))
        chunks = ctx.enter_context(tc.tile_pool(name="chunks", bufs=10))
        psumg = ctx.enter_context(tc.tile_pool(name="psumg", bufs=1, space="PSUM"))
        outpp = ctx.enter_context(tc.tile_pool(name="outpp", bufs=2, space="PSUM"))
        psums = ctx.enter_context(tc.tile_pool(name="psums", bufs=1, space="PSUM"))
        red = ctx.enter_context(tc.tile_pool(name="red", bufs=2))
        eig = ctx.enter_context(tc.tile_pool(name="eig", bufs=2))
        plp = ctx.enter_context(tc.tile_pool(name="plp", bufs=3))

        auxp = const.tile([128, AUXW], F32)
        nc.sync.dma_start(auxp[:], bass.AP(aux, 0, [[AUXW, 128], [1, AUXW]]))
        sttile = const.tile([1, BPC * 20], F32)

        def emit_gram_half(t, h2, first, last):
            psA = state["psA"]
            psB = state["psB"]
            for i in range(8):
                sb = h2 * 8 + i
                rhs = t[:, sb * 256:(sb + 1) * 256]
                lhsA = t[:, sb * 256:sb * 256 + 128]
                lhsB = t[:, sb * 256 + 128:(sb + 1) * 256]
                st0 = first and i == 0
                sp = last and i == 7
                nc.tensor.matmul(psA[:], lhsA, rhs, start=st0, stop=sp)
                nc.tensor.matmul(psB[:], lhsB, rhs, start=st0, stop=sp)

        def emit_extract_eigen(b, psA, psB):
            maskA = auxp[:, C_MA:C_MA + 256]
            maskB = auxp[:, C_MB:C_MB + 256]
            dm16 = auxp[:, C_DM:C_DM + 16]
            dm16q = auxp[:, C_DMQ:C_DMQ + 16]
            # ---- extraction: S = sum_g G256[4g+k, 4g+l] ----
            mA = red.tile([128, 256], F32, tag="mA")
            nc.vector.tensor_mul(mA[:], psA[:], maskA)
            mB = red.tile([128, 256], F32, tag="mB")
            nc.vector.tensor_mul(mB[:], psB[:], maskB)
            m4A = red.tile([128, 4], F32, tag="m4A")
            nc.vector.tensor_reduce(m4A[:], _v(mA[:], [[1, 4], [4, 64]]),
                                    AXL.X, ALU.add)
            m4B = red.tile([128, 4], F32, tag="m4B")
            nc.vector.tensor_reduce(m4B[:], _v(mB[:], [[1, 4], [4, 64]]),
                                    AXL.X, ALU.add)
            psE = psums.tile([4, 4], F32, tag="scr", name=f"psE_{b}")
            E4 = auxp[:, C_E4:C_E4 + 4]
            nc.tensor.matmul(psE[:], E4, m4A[:], start=True, stop=False)
            nc.tensor.matmul(psE[:], E4, m4B[:], start=False, stop=True)
            Fm16 = red.tile([4, 16], F32, tag="Fm16")
            s_b = _v(psE[:], [[0, 4], [1, 4]])
            nc.vector.tensor_tensor(Fm16[:].rearrange("p (j l) -> p j l", j=4),
                                    s_b,
                                    _v(auxp[0:4, :], [[4, 4], [1, 4]], C_FM),
                                    ALU.mult)
            psS = psums.tile([128, 16], F32, tag="psS", name=f"psS_{b}")
            nc.tensor.matmul(psS[:], auxp[0:4, C_ONES:C_ONES + 128], Fm16[:],
                             start=True, stop=True)

            # ---- eigen setup (replicated flat [128, 16]) ----
            covf = eig.tile([128, 16], F32, tag="covf")
            nc.vector.scalar_tensor_tensor(
                covf[:], psS[:], 1.0 / NROWS,
                auxp[:, MMF_OFF + 16 * b:MMF_OFF + 16 * b + 16],
                ALU.mult, ALU.subtract)
            trqn = eig.tile([128, 1], F32, tag="trqn")
            nc.vector.tensor_reduce(trqn[:], _v(covf[:], [[5, 4]]), AXL.X,
                                    ALU.add, negate=True)
            B0 = eig.tile([128, 16], F32, tag="B0")
            nc.vector.scalar_tensor_tensor(B0[:], dm16q, trqn[:], covf[:],
                                           ALU.mult, ALU.add)
            absr = eig.tile([128, 4], F32, tag="absr")
            nc.vector.tensor_reduce(absr[:].rearrange("p (i u) -> p i u", i=4),
                                    B0[:].rearrange("p (i j) -> p i j", i=4),
                                    AXL.X, ALU.add, apply_absolute_value=True)
            rsh = eig.tile([128, 1], F32, tag="rsh")
            nc.vector.tensor_reduce(rsh[:], absr[:], AXL.X, ALU.max)
            rrec = eig.tile([128, 1], F32, tag="rrec")
            nc.vector.reciprocal(rrec[:], rsh[:])
            Bc = eig.tile([128, 16], F32, tag="Bc")
            nc.vector.scalar_tensor_tensor(Bc[:], dm16, rsh[:], B0[:],
                                           ALU.mult, ALU.add)
            nc.vector.tensor_scalar(Bc[:], Bc[:], rrec[:], 0.5, ALU.mult,
                                    ALU.mult)
            # ---- squarings as [4,4] PE matmuls ----
            Bpe = eig.tile([4, 4], F32, tag="Bpe")
            nc.gpsimd.dma_start(Bpe[:], _v(Bc[0:1, :], [[1, 16]]))
            cur = Bpe
            for k in range(NSQ):
                psq = psums.tile([4, 4], F32, tag="scr", name=f"psq_{b}_{k}")
                nc.tensor.matmul(psq[:], cur[:], cur[:], start=True, stop=True)
                nxt = eig.tile([4, 4], F32, tag=f"sq{k % 2}")
                nc.scalar.copy(nxt[:], psq[:])
                cur = nxt
            # ---- v = C @ e, replicate across partitions via PE ----
            vps = psums.tile([4, 1], F32, tag="scr", name=f"vps_{b}")
            nc.tensor.matmul(vps[:], cur[:], auxp[0:4, C_E:C_E + 1],
                             start=True, stop=True)
            vdiag = eig.tile([4, 4], F32, tag="vdiag")
            nc.vector.tensor_tensor(vdiag[:], _v(vps[:], [[0, 4]]),
                                    auxp[0:4, C_I4:C_I4 + 4], ALU.mult)
            vrps = psums.tile([128, 4], F32, tag="scr", name=f"vrps_{b}")
            nc.tensor.matmul(vrps[:], auxp[0:4, C_ONES:C_ONES + 128],
                             vdiag[:], start=True, stop=True)
            v_rep = eig.tile([128, 4], F32, tag="v_rep")
            nc.scalar.copy(v_rep[:], vrps[:])
            # stats slices (batched DMA after last sample)
            nc.scalar.copy(sttile[:, 20 * b:20 * b + 16], psS[0:1, :])
            nc.scalar.copy(sttile[:, 20 * b + 16:20 * b + 20], v_rep[0:1, :])
            wks = []
            for k in range(4):
                wk = eig.tile([128, 128], F32R, tag=f"wk{k}", name=f"wk{b}_{k}")
                nc.vector.tensor_scalar(wk[:], auxp[:, C_I128:C_I128 + 128],
                                        v_rep[:, k:k + 1], None, ALU.mult)
                wks.append(wk)
            return wks

        def emit_proj(pb, pctiles, wks):
            for ci, t in enumerate(pctiles):
                half, q = divmod(ci, 2)
                outp = outpp.tile([128, 1024], F32, tag="outp",
                                  name=f"op{pb}_{ci}")
                for h in range(2):
                    for k in range(4):
                        rhs = _v(t[:], [[512, 4], [4, 128]],
                                 extra_off=k + h * 2048)
                        nc.tensor.matmul(outp[:, 512 * h:512 * h + 512],
                                         wks[k][:], rhs,
                                         start=(k == 0), stop=(k == 3))
                pa = plp.tile([128, 1024], F32, tag="pa", name=f"pa{pb}_{ci}")
                nc.vector.tensor_copy(pa[:], outp[:])
                nc.scalar.dma_start(
                    bass.AP(y, pb * OUT_SAMPLE + q * 2048 + half * 256,
                            [[4096, 128], [512, 4], [1, 256]]),
                    pa[:])

        state = {}
        prev = None
        for b in range(BPC):
            ctiles = []
            state["psA"] = psumg.tile([128, 256], F32, tag="psA",
                                      name=f"psA_{b}")
            state["psB"] = psumg.tile([128, 256], F32, tag="psB",
                                      name=f"psB_{b}")
            dmas = []
            for ci in range(4):
                half, q = divmod(ci, 2)
                t = chunks.tile([128, 4096], F32R, tag="chunk",
                                name=f"t_{b}_{ci}")
                ctiles.append(t)
                for h2 in range(2):
                    nc.sync.dma_start(
                        t[:, h2 * 2048:(h2 + 1) * 2048],
                        _in_dram_ap_half(x, b, half, q, h2).bitcast(F32R))
            if prev is not None:
                emit_proj(*prev)
            for ci in range(4):
                for h2 in range(2):
                    emit_gram_half(ctiles[ci], h2,
                                   first=(ci == 0 and h2 == 0),
                                   last=(ci == 3 and h2 == 1))
            wks = emit_extract_eigen(b, state["psA"], state["psB"])
            prev = (b, ctiles, wks)
        emit_proj(*prev)
        nc.scalar.dma_start(bass.AP(st, 0, [[BPC * 20, 1], [1, BPC * 20]]),
                            sttile[:])
    nc.compile()
    return nc


_CACHE = {}


def _get(name, builder):
    if name not in _CACHE:
        _CACHE[name] = builder()
    return _CACHE[name]


def make_aux(mean):
    """mean: [BPC, 4] float -> aux array [128, AUXW]."""
    a = np.zeros((128, AUXW), np.float32)
    p = np.arange(128)
    for b in range(BPC):
        mm = np.outer(mean[b], mean[b]).astype(np.float32).reshape(16)
        a[:, MMF_OFF + 16 * b:MMF_OFF + 16 * b + 16] = mm
    a[0:4, C_E] = np.asarray(EVEC, np.float32)
    a[0:4, C_I4:C_I4 + 4] = np.eye(4, dtype=np.float32)
    u = np.arange(16)
    a[0:4, C_FM:C_FM + 16] = (u[None, :] // 4 == np.arange(4)[:, None])
    a[:, C_DM:C_DM + 16] = ((u // 4) == (u % 4)).astype(np.float32)[None, :]
    a[:, C_DMQ:C_DMQ + 16] = 0.25 * a[:, C_DM:C_DM + 16]
    a[:, C_E4:C_E4 + 4] = (np.arange(4)[None, :] == (p % 4)[:, None])
    a[:, C_ONES:C_ONES + 128] = 1.0
    n = np.arange(256)
    a[:, C_MA:C_MA + 256] = ((n[None, :] >> 2) == (p >> 2)[:, None])
    a[:, C_MB:C_MB + 256] = ((n[None, :] >> 2) == 32 + (p >> 2)[:, None])
    a[:, C_I128:C_I128 + 128] = np.eye(128, dtype=np.float32)
    return a


def kernel(inputs: np.ndarray) -> np.ndarray:
    xx = np.ascontiguousarray(np.asarray(inputs, dtype=np.float32))
    assert xx.shape == (B, H, W, C), xx.shape
    xf = xx.reshape(N_CORES, BPC * SAMPLE)
    cores = list(range(N_CORES))
    mean = xx.reshape(B, NROWS, 4).mean(axis=1, dtype=np.float64)  # [B, 4]

    nc = _get("fused", _build_fused)
    in_maps = [
        {"x": xf[c], "aux": make_aux(mean[c * BPC:(c + 1) * BPC])} for c in cores
    ]
    r = run_bass_kernel_spmd(nc, in_maps, cores)
    stats = np.stack([r.results[c]["stats"] for c in cores]).reshape(B, 20)
    yv = np.stack([r.results[c]["y"] for c in cores]).reshape(B, OUT_SAMPLE)

    S = stats[:, 0:16].reshape(B, 4, 4).astype(np.float64)
    v_dev = stats[:, 16:20].astype(np.float64)
    cov = (S / NROWS - np.einsum("bi,bj->bij", mean, mean)).astype(np.float32)

    import jax
    import jax.numpy as jnp
    with jax.default_device(jax.devices("cpu")[0]):
        _, vecs = jnp.linalg.eigh(jnp.asarray(cov))
    v_ref = np.asarray(vecs)[:, :, -1].astype(np.float64)

    dot = (v_ref * v_dev).sum(1)
    scale = np.sign(dot) / np.linalg.norm(v_dev, axis=1)
    offs = -(mean * v_dev).sum(1) * scale          # fold -mu.v into host
    yv = (yv * scale[:, None] + offs[:, None]).astype(np.float32)
    return yv.reshape(B, H // 2, W // 2, C)
